# revision 18
# baseline (speedup 1.0000x reference)
"""Trainium2 Bass kernel for nn_Attention_84679575208344 (Performer-style
linear attention). Data-parallel over batch: 8 batches -> 8 NeuronCores.

Math per batch b (reference):
  qkv = x @ Wqkv.T -> split q,k,v per head (HD=48)
  qp = relu(dn*q)+1e-3 ; kp = relu(dn*k)+1e-3          (dn = 48**-0.25)
  ks = kp.sum(n) ; D = qp @ ks ; kptv = v.T @ kp (per head)
  attn = (qp @ kptv.T) / (D + 1e-8)
  out  = reshape(B,H,N,HD)->(B,N,C) WITHOUT head transpose, then @ Wproj.T + b

The no-transpose reshape means output row n' = 512*h + q holds
attn[h, 8q+j, d] at column 48j+d. We compute attention transposed
(features on partitions), build A^T[c''=64j+d, q] directly, and do the
projection with a head-padded Wproj^T (zero rows kill the padding).
"""

from contextlib import ExitStack

import numpy as np

import concourse.bass as bass
import concourse.mybir as mybir
import concourse.tile as tile
from concourse import bacc

F32 = mybir.dt.float32
F32R = mybir.dt.float32r
AL = mybir.AluOpType
FCOPY = mybir.ActivationFunctionType.Copy
FID = mybir.ActivationFunctionType.Identity

B, N, C, H = 8, 4096, 384, 8
HD = 48
KEPS = 1e-3
EPS = 1e-8
DN = float(HD ** (-0.25))
NCHUNK = N // 128  # 32
NBLK = N // 512    # 8

_NC_CACHE = {}


def _rep_row(src_ap, n):
    """Replicate a [1, F] SBUF row AP to n rows via a zero-step middle dim."""
    return bass.AP(tensor=src_ap.tensor, offset=src_ap.offset,
                   ap=[src_ap.ap[0], [0, n], src_ap.ap[1]])


def build_nc():
    nc = bacc.Bacc("TRN2", target_bir_lowering=False, debug=False, num_devices=8)
    x = nc.declare_dram_parameter("x", [N, C], F32, isOutput=False)
    wq = nc.declare_dram_parameter("wq", [C, 512], F32, isOutput=False)
    wkv = nc.declare_dram_parameter("wkv", [C, 768], F32, isOutput=False)
    wp = nc.declare_dram_parameter("wp", [512, C], F32, isOutput=False)
    bias = nc.declare_dram_parameter("bias", [C], F32, isOutput=False)
    ident_d = nc.declare_dram_parameter("ident", [128, 128], F32, isOutput=False)
    out = nc.declare_dram_parameter("out", [N, C], F32, isOutput=True)

    with tile.TileContext(nc) as tc, ExitStack() as ctx:
        persist = ctx.enter_context(tc.tile_pool(name="persist", bufs=1))
        xin_p = ctx.enter_context(tc.tile_pool(name="xin", bufs=3))
        kp_p = ctx.enter_context(tc.tile_pool(name="kp", bufs=2))
        v_p = ctx.enter_context(tc.tile_pool(name="v", bufs=2))
        rd_p = ctx.enter_context(tc.tile_pool(name="rd", bufs=8))
        rb_p = ctx.enter_context(tc.tile_pool(name="rb", bufs=8))
        at_p = ctx.enter_context(tc.tile_pool(name="at", bufs=2))
        ab_p = ctx.enter_context(tc.tile_pool(name="ab", bufs=2))
        zo_p = ctx.enter_context(tc.tile_pool(name="zo", bufs=3))

        xT = persist.tile([128, 3, N], F32R)      # x^T: chunk kc = c rows 128kc..+128
        qpT = persist.tile([128, 4, N], F32R)     # padded qp^T: head h at [64*(h%2)+d, h//2]
        wq_sb = persist.tile([128, 3, 512], F32R)
        wkv_sb = persist.tile([128, 3, 768], F32R)
        wp_sb = persist.tile([128, 4, C], F32R)
        ident = persist.tile([128, 128], F32)
        ones82_f = persist.tile([128, 8, 2], F32)
        ones82 = persist.tile([128, 8, 2], F32R)
        kptv_sb = persist.tile([128, 4, 49], F32R)  # [m(+64 for odd h), h//2, d|ks]
        ks_f = persist.tile([128, 4, 8], F32)
        ks_sb = persist.tile([128, 4, 8], F32R)
        row_mask = persist.tile([128, 1], F32)

        nc.sync.dma_start(out=ident[:], in_=ident_d[:])
        nc.gpsimd.dma_start(out=wkv_sb[:], in_=wkv[:].rearrange("(c p) d -> p c d", p=128))
        nc.gpsimd.dma_start(out=wq_sb[:], in_=wq[:].rearrange("(c p) d -> p c d", p=128))
        nc.gpsimd.dma_start(out=wp_sb[:], in_=wp[:].rearrange("(c p) d -> p c d", p=128))
        nc.vector.memset(ones82_f[:], 1.0)
        nc.vector.tensor_copy(out=ones82[:], in_=ones82_f[:])
        nc.vector.memset(ks_f[:], 0.0)
        nc.vector.memset(row_mask[:], 0.0)
        ocell = ones82_f[0:1, 0, 0:1]
        nc.sync.dma_start(out=row_mask[63:64, 0:1],
                          in_=bass.AP(tensor=ocell.tensor, offset=ocell.offset,
                                      ap=[list(ocell.ap[0]), [1, 1]]))

        # ---------------- phase 1: x^T, K/V, kptv, qp^T ----------------
        with tc.tile_pool(name="ptrq", bufs=3, space="PSUM") as ptrq_p, \
             tc.tile_pool(name="pkv", bufs=2, space="PSUM") as pkv_p, \
             tc.tile_pool(name="pkp", bufs=1, space="PSUM") as pkp_p:
            psum_kptv = pkp_p.tile([48, 8, 50], F32)

            def emit_q_block(blk):
                bs = slice(512 * blk, 512 * (blk + 1))
                for mc in range(4):
                    pq = ptrq_p.tile([128, 512], F32, tag="ptrq")
                    for kc in range(3):
                        nc.tensor.matmul(pq[:], wq_sb[:, kc, 128 * mc:128 * (mc + 1)],
                                         xT[:, kc, bs],
                                         start=(kc == 0), stop=(kc == 2))
                    nc.vector.tensor_scalar(qpT[:, mc, bs], pq[:], 0.0, KEPS,
                                            op0=AL.max, op1=AL.add)

            for i in range(NCHUNK):
                ns = slice(128 * i, 128 * (i + 1))
                xin = xin_p.tile([128, C], F32)
                nc.scalar.dma_start(out=xin[:], in_=x[ns, :])
                ptr = ptrq_p.tile([128, 512], F32, tag="ptrq")
                for kc in range(3):
                    nc.tensor.matmul(ptr[:, 128 * kc:128 * (kc + 1)],
                                     xin[:, 128 * kc:128 * (kc + 1)], ident[:],
                                     is_transpose=True, start=True, stop=True)
                for kc in range(3):
                    nc.scalar.copy(out=xT[:, kc, ns],
                                   in_=ptr[:, 128 * kc:128 * (kc + 1)])
                pkv = pkv_p.tile([128, 768], F32)
                for kc in range(3):
                    lhs = xT[:, kc, ns]
                    nc.tensor.matmul(pkv[:, 0:512], lhs, wkv_sb[:, kc, 0:512],
                                     start=(kc == 0), stop=(kc == 2))
                    nc.tensor.matmul(pkv[:, 512:768], lhs, wkv_sb[:, kc, 512:768],
                                     start=(kc == 0), stop=(kc == 2))
                kp = kp_p.tile([128, C], F32R)
                nc.vector.tensor_scalar(kp[:], pkv[:, 0:C], 0.0, KEPS,
                                        op0=AL.max, op1=AL.add)
                v = v_p.tile([128, 8, 50], F32R)
                nc.scalar.copy(
                    out=v[:, :, 0:48],
                    in_=pkv[:, C:768].rearrange("p (h d) -> p h d", h=8))
                nc.vector.tensor_copy(out=v[:, :, 48:50], in_=ones82[:])
                for h in range(H):
                    nc.tensor.matmul(psum_kptv[:, h, :], kp[:, 48 * h:48 * (h + 1)],
                                     v[:, h, :],
                                     start=(i == 0 and h == 0),
                                     stop=(i == NCHUNK - 1 and h == H - 1))
                if i % 4 == 0 and i > 0:
                    emit_q_block(i // 4 - 1)
            emit_q_block(NBLK - 1)

            # kptv psum -> sbuf, then DMA-remap heads to their qpT partition homes
            kptv_tmp = persist.tile([48, 8, 49], F32R)
            nc.vector.tensor_copy(out=kptv_tmp[:], in_=psum_kptv[:, :, 0:49])
            nc.sync.dma_start(out=kptv_sb[0:48, :, :], in_=kptv_tmp[:, 0::2, :])
            nc.sync.dma_start(out=kptv_sb[64:112, :, :], in_=kptv_tmp[:, 1::2, :])
            for h in range(H):
                p0 = 64 * (h % 2)
                nc.vector.tensor_copy(out=ks_f[p0:p0 + 48, h // 2, h:h + 1],
                                      in_=kptv_sb[p0:p0 + 48, h // 2, 48:49])
            nc.vector.tensor_copy(out=ks_sb[:], in_=ks_f[:])

        # ---------------- phase 2+3: D, attention, projection ----------------
        with tc.tile_pool(name="po", bufs=4, space="PSUM") as po_p, \
             tc.tile_pool(name="pd", bufs=1, space="PSUM") as pd_p, \
             tc.tile_pool(name="pz", bufs=3, space="PSUM") as pz_p:
            rds = []
            for j in range(8):
                pd = pd_p.tile([8, 512], F32)
                for cc in range(4):
                    rhs = qpT[:, cc, :].rearrange("p (r j) -> p j r", j=8)[:, j, :]
                    nc.tensor.matmul(pd[:], ks_sb[:, cc, :], rhs,
                                     start=(cc == 0), stop=(cc == 3))
                rd = rd_p.tile([8, 512], F32)
                nc.vector.tensor_scalar_add(rd[:], pd[:], EPS)
                nc.vector.reciprocal(rd[:], rd[:])
                rds.append(rd)

            def emit_attn_head(h):
                p0 = 64 * (h % 2)
                at = at_p.tile([128, 4, 512], F32R)
                zsrc = wkv_sb[:, :, :].rearrange("p a b -> p (a b)")
                nc.scalar.activation(
                    at[32:64, :, :].rearrange("p a b -> p (a b)"),
                    zsrc[32:64, 0:2048], FID,
                    bias=row_mask[32:64, :], scale=0.0)
                nc.scalar.activation(
                    at[96:128, :, :].rearrange("p a b -> p (a b)"),
                    zsrc[96:128, 0:2048], FCOPY, bias=0.0, scale=0.0)
                qh = qpT[p0:p0 + 48, h // 2, :].rearrange("p (r j) -> p j r", j=8)
                for j in range(8):
                    rb = rb_p.tile([48, 512], F32)
                    dma_eng = nc.sync if j % 2 == 0 else nc.scalar
                    dma_eng.dma_start(out=rb[:], in_=_rep_row(rds[j][h:h + 1, :], 48))
                    po = po_p.tile([48, 512], F32)
                    nc.tensor.matmul(po[:], kptv_sb[p0:p0 + 48, h // 2, 0:48],
                                     qh[:, j, :], start=True, stop=True,
                                     tile_position=(p0, 0))
                    if j % 2 == 0:
                        nc.vector.tensor_mul(at[0:48, j // 2, :], po[:], rb[:])
                    else:
                        ab = ab_p.tile([48, 512], F32R)
                        nc.vector.tensor_mul(ab[:], po[:], rb[:])
                        reng = (nc.gpsimd, nc.sync, nc.scalar, nc.gpsimd)[(j // 2) % 4]
                        reng.dma_start(out=at[64:112, j // 2, :], in_=ab[:])
                return at

            def emit_proj_head(h, at):
                for rc in range(4):
                    pz = pz_p.tile([128, C], F32)
                    for cc in range(4):
                        nc.tensor.matmul(pz[:], at[:, cc, 128 * rc:128 * (rc + 1)],
                                         wp_sb[:, cc, :],
                                         start=(cc == 0), stop=(cc == 3))
                    zo = zo_p.tile([128, C], F32)
                    nc.scalar.copy(out=zo[:], in_=pz[:])
                    r0 = 512 * h + 128 * rc
                    nc.sync.dma_start(out=out[r0:r0 + 128, :], in_=zo[:])

            ats = {}
            ats[0] = emit_attn_head(0)
            ats[1] = emit_attn_head(1)
            for h in range(H):
                if h + 2 < H:
                    ats[h + 2] = emit_attn_head(h + 2)
                emit_proj_head(h, ats.pop(h))
    nc.finalize()
    return nc


def _prep_weights(Wqkv, Wproj, bproj=None):
    """Host-side weight prep: fold dn, pad head dims, build transposed layouts."""
    Wq = Wqkv[0:C, :]
    Wk = Wqkv[C:2 * C, :]
    Wv = Wqkv[2 * C:3 * C, :]
    wq = np.zeros((C, 512), np.float32)
    for h in range(H):
        wq[:, 64 * h:64 * h + 48] = (DN * Wq[48 * h:48 * (h + 1), :]).T
    wkv = np.concatenate([(DN * Wk).T, Wv.T], axis=1).astype(np.float32)
    wp = np.zeros((512, C), np.float32)
    WprojT = Wproj.T
    for j in range(8):
        wp[64 * j:64 * j + 48, :] = WprojT[48 * j:48 * (j + 1), :]
    if bproj is not None:
        wp[63, :] = bproj
    return wq, wkv, wp


def _run(inputs, trace=False):
    from concourse.bass_utils import run_bass_kernel_spmd

    x = np.ascontiguousarray(np.asarray(inputs["x"], dtype=np.float32))
    Wqkv = np.asarray(inputs["Wqkv"], dtype=np.float32)
    Wproj = np.asarray(inputs["Wproj"], dtype=np.float32)
    bproj = np.ascontiguousarray(np.asarray(inputs["bproj"], dtype=np.float32))
    wq, wkv, wp = _prep_weights(Wqkv, Wproj, bproj)

    if "nc" not in _NC_CACHE:
        _NC_CACHE["nc"] = build_nc()
    nc = _NC_CACHE["nc"]

    ident = np.eye(128, dtype=np.float32)
    in_maps = [
        {"x": np.ascontiguousarray(x[b]), "wq": wq, "wkv": wkv, "wp": wp,
         "bias": bproj, "ident": ident}
        for b in range(B)
    ]
    res = run_bass_kernel_spmd(nc, in_maps, list(range(8)), trace=trace)
    out = np.stack([res.results[b]["out"] for b in range(B)], axis=0)
    return out, res


def kernel(**inputs) -> np.ndarray:
    out, _ = _run(inputs, trace=False)
    return out


def kernel_profiled(**inputs):
    out, res = _run(inputs, trace=True)
    return out, res


# revision 34
# speedup vs baseline: 50.0458x; 50.0458x over previous
"""Trainium2 Bass kernel for nn_Attention_84679575208344 (Performer-style
linear attention). Data-parallel over batch: 8 batches -> 8 NeuronCores.

Math per batch b (reference):
  qkv = x @ Wqkv.T -> split q,k,v per head (HD=48)
  qp = relu(dn*q)+1e-3 ; kp = relu(dn*k)+1e-3          (dn = 48**-0.25)
  ks = kp.sum(n) ; D = qp @ ks ; kptv = v.T @ kp (per head)
  attn = (qp @ kptv.T) / (D + 1e-8)
  out  = reshape(B,H,N,HD)->(B,N,C) WITHOUT head transpose, then @ Wproj.T + b

The no-transpose reshape means output row n' = 512*h + q holds
attn[h, 8q+j, d] at column 48j+d. We compute attention transposed
(features on partitions), build A^T[c''=64j+d, q] directly, and do the
projection with a head-padded Wproj^T (zero rows kill the padding).
"""

from contextlib import ExitStack

import numpy as np

import concourse.bass as bass
import concourse.mybir as mybir
import concourse.tile as tile
from concourse import bacc

F32 = mybir.dt.float32
F32R = mybir.dt.float32r
BF16 = mybir.dt.bfloat16
AL = mybir.AluOpType
FCOPY = mybir.ActivationFunctionType.Copy
FID = mybir.ActivationFunctionType.Identity

B, N, C, H = 8, 4096, 384, 8
HD = 48
KEPS = 1e-3
EPS = 1e-8
DN = float(HD ** (-0.25))
NCHUNK = N // 128  # 32
NBLK = N // 512    # 8

_NC_CACHE = {}


def _rep_row(src_ap, n):
    """Replicate a [1, F] SBUF row AP to n rows via a zero-step middle dim."""
    return bass.AP(tensor=src_ap.tensor, offset=src_ap.offset,
                   ap=[src_ap.ap[0], [0, n], src_ap.ap[1]])


def build_nc():
    nc = bacc.Bacc("TRN2", target_bir_lowering=False, debug=False, num_devices=8)
    x = nc.declare_dram_parameter("x", [N, C], F32, isOutput=False)
    wq = nc.declare_dram_parameter("wq", [C, 512], F32, isOutput=False)
    wkv = nc.declare_dram_parameter("wkv", [C, 768], F32, isOutput=False)
    wp = nc.declare_dram_parameter("wp", [512, C], F32, isOutput=False)
    bias = nc.declare_dram_parameter("bias", [C], F32, isOutput=False)
    ident_d = nc.declare_dram_parameter("ident", [128, 128], F32, isOutput=False)
    out = nc.declare_dram_parameter("out", [N, C], F32, isOutput=True)

    with tile.TileContext(nc) as tc, ExitStack() as ctx:
        persist = ctx.enter_context(tc.tile_pool(name="persist", bufs=1))
        xin_p = ctx.enter_context(tc.tile_pool(name="xin", bufs=3))
        kp_p = ctx.enter_context(tc.tile_pool(name="kp", bufs=2))
        v_p = ctx.enter_context(tc.tile_pool(name="v", bufs=2))
        rbig_p = ctx.enter_context(tc.tile_pool(name="rbig", bufs=4))
        ab_p = ctx.enter_context(tc.tile_pool(name="ab", bufs=2))
        zo_p = ctx.enter_context(tc.tile_pool(name="zo", bufs=3))

        qpT = persist.tile([128, 4, N], F32R)     # padded qp^T: head h at [64*(h%2)+d, h//2]
        wq_sb = persist.tile([128, 3, 512], F32R)
        wkv_sb = persist.tile([128, 3, 768], F32R)
        wp_sb = persist.tile([128, 4, C], F32R)
        ident = persist.tile([128, 128], F32)
        ones82 = persist.tile([128, 8, 2], BF16)
        kptv_sb = persist.tile([128, 4, 49], F32R)  # [m(+64 for odd h), h//2, d|ks]
        ks_f = persist.tile([128, 4, 8], F32)
        ks_sb = persist.tile([128, 4, 8], F32R)
        row_mask = persist.tile([128, 1], F32)

        nc.sync.dma_start(out=ident[:], in_=ident_d[:])
        nc.gpsimd.dma_start(out=wkv_sb[:], in_=wkv[:].rearrange("(c p) d -> p c d", p=128))
        nc.gpsimd.dma_start(out=wq_sb[:], in_=wq[:].rearrange("(c p) d -> p c d", p=128))
        nc.gpsimd.dma_start(out=wp_sb[:], in_=wp[:].rearrange("(c p) d -> p c d", p=128))
        nc.vector.memset(ones82[:], 1.0)
        nc.vector.memset(ks_f[:], 0.0)
        nc.vector.memset(row_mask[:], 0.0)
        one_f = persist.tile([1, 1], F32)
        nc.vector.memset(one_f[:], 1.0)
        ocell = one_f[0:1, 0:1]
        nc.sync.dma_start(out=row_mask[63:64, 0:1], in_=ocell)

        # ---------------- phase 1: x^T, K/V, kptv, qp^T ----------------
        with tc.tile_pool(name="ptrq", bufs=3, space="PSUM") as ptrq_p, \
             tc.tile_pool(name="pkv", bufs=2, space="PSUM") as pkv_p, \
             tc.tile_pool(name="pkp", bufs=1, space="PSUM") as pkp_p, \
             tc.tile_pool(name="xtp", bufs=1) as xt_p:
            psum_kptv = pkp_p.tile([48, 8, 50], F32)
            xT = xt_p.tile([128, 3, N], F32R)  # x^T; dies with phase 1

            def emit_q_block(blk):
                bs = slice(512 * blk, 512 * (blk + 1))
                for mc in range(4):
                    pq = ptrq_p.tile([128, 512], F32, tag="ptrq")
                    for kc in range(3):
                        nc.tensor.matmul(pq[:], wq_sb[:, kc, 128 * mc:128 * (mc + 1)],
                                         xT[:, kc, bs],
                                         start=(kc == 0), stop=(kc == 2))
                    nc.vector.tensor_scalar(qpT[:, mc, bs], pq[:], 0.0, KEPS,
                                            op0=AL.max, op1=AL.add)

            for i in range(NCHUNK):
                ns = slice(128 * i, 128 * (i + 1))
                xin = xin_p.tile([128, C], F32)
                nc.scalar.dma_start(out=xin[:], in_=x[ns, :])
                ptr = ptrq_p.tile([128, 512], F32, tag="ptrq")
                for kc in range(3):
                    nc.tensor.matmul(ptr[:, 128 * kc:128 * (kc + 1)],
                                     xin[:, 128 * kc:128 * (kc + 1)], ident[:],
                                     is_transpose=True, start=True, stop=True)
                for kc in range(3):
                    nc.scalar.copy(out=xT[:, kc, ns],
                                   in_=ptr[:, 128 * kc:128 * (kc + 1)])
                pkv = pkv_p.tile([128, 768], F32)
                for kc in range(3):
                    lhs = xT[:, kc, ns]
                    nc.tensor.matmul(pkv[:, 0:512], lhs, wkv_sb[:, kc, 0:512],
                                     start=(kc == 0), stop=(kc == 2))
                    nc.tensor.matmul(pkv[:, 512:768], lhs, wkv_sb[:, kc, 512:768],
                                     start=(kc == 0), stop=(kc == 2))
                kp = kp_p.tile([128, C], BF16)
                nc.vector.tensor_scalar(kp[:], pkv[:, 0:C], 0.0, KEPS,
                                        op0=AL.max, op1=AL.add)
                v = v_p.tile([128, 8, 50], BF16)
                nc.scalar.copy(
                    out=v[:, :, 0:48],
                    in_=pkv[:, C:768].rearrange("p (h d) -> p h d", h=8))
                nc.vector.tensor_copy(out=v[:, :, 48:50], in_=ones82[:])
                for h in range(H):
                    nc.tensor.matmul(psum_kptv[:, h, :], kp[:, 48 * h:48 * (h + 1)],
                                     v[:, h, :],
                                     start=(i == 0 and h == 0),
                                     stop=(i == NCHUNK - 1 and h == H - 1))
                if i % 4 == 0 and i > 0:
                    emit_q_block(i // 4 - 1)
            emit_q_block(NBLK - 1)

            # kptv psum -> sbuf, then DMA-remap heads to their qpT partition homes
            kptv_tmp = persist.tile([48, 8, 49], F32R)
            nc.vector.tensor_copy(out=kptv_tmp[:], in_=psum_kptv[:, :, 0:49])
            nc.sync.dma_start(out=kptv_sb[0:48, :, :], in_=kptv_tmp[:, 0::2, :])
            nc.sync.dma_start(out=kptv_sb[64:112, :, :], in_=kptv_tmp[:, 1::2, :])
            for h in range(H):
                p0 = 64 * (h % 2)
                nc.vector.tensor_copy(out=ks_f[p0:p0 + 48, h // 2, h:h + 1],
                                      in_=kptv_sb[p0:p0 + 48, h // 2, 48:49])
            nc.vector.tensor_copy(out=ks_sb[:], in_=ks_f[:])

        # ---------------- phase 2+3: D, attention, projection ----------------
        tc.strict_bb_all_engine_barrier()
        with tc.tile_pool(name="po", bufs=3, space="PSUM") as po_p, \
             tc.tile_pool(name="pd", bufs=2, space="PSUM") as pd_p, \
             tc.tile_pool(name="pz", bufs=3, space="PSUM") as pz_p, \
             tc.tile_pool(name="p23", bufs=1) as p23_p:
            rd_all = p23_p.tile([8, N], F32)
            at0 = p23_p.tile([128, 4, 512], F32R, tag="at0")
            at1 = p23_p.tile([128, 4, 512], F32R, tag="at1")
            zsrc = wkv_sb[:, :, :].rearrange("p a b -> p (a b)")
            for at in (at0, at1):
                nc.scalar.activation(
                    at[32:64, :, :].rearrange("p a b -> p (a b)"),
                    zsrc[32:64, 0:2048], FID,
                    bias=row_mask[32:64, :], scale=0.0)
                nc.scalar.activation(
                    at[96:128, :, :].rearrange("p a b -> p (a b)"),
                    zsrc[96:128, 0:2048], FCOPY, bias=0.0, scale=0.0)

            rdj = rd_all[:].rearrange("p (r j) -> p j r", j=8)
            # D matmuls interleaved with heads 0/1 on the unnormalized path:
            # their attention matmuls + explicit 1/D muls fill the PE pipeline
            # while D/recips for the remaining heads are still being computed.
            qh01 = [qpT[64 * hh:64 * hh + 48, 0, :].rearrange("p (r j) -> p j r", j=8)
                    for hh in range(2)]
            for j in range(8):
                pd = pd_p.tile([8, 512], F32)
                for cc in range(4):
                    rhs = qpT[:, cc, :].rearrange("p (r j) -> p j r", j=8)[:, j, :]
                    nc.tensor.matmul(pd[:], ks_sb[:, cc, :], rhs,
                                     start=(cc == 0), stop=(cc == 3))
                rcj = rbig_p.tile([8, 512], F32, tag="rcj")
                nc.vector.tensor_scalar_add(rcj[:], pd[:], EPS)
                nc.vector.reciprocal(rcj[:], rcj[:])
                nc.vector.tensor_copy(out=rdj[:, j, :], in_=rcj[:])
                for hh in range(2):
                    p0 = 64 * hh
                    at = at0 if hh == 0 else at1
                    po = po_p.tile([48, 512], F32)
                    nc.tensor.matmul(po[:], kptv_sb[p0:p0 + 48, 0, 0:48],
                                     qh01[hh][:, j, :], start=True, stop=True,
                                     tile_position=(p0, 0))
                    rb = rbig_p.tile([48, 512], F32, tag="rb")
                    deng = nc.sync if (j + hh) % 2 == 0 else nc.scalar
                    deng.dma_start(out=rb[:], in_=_rep_row(rcj[hh:hh + 1, :], 48))
                    if j % 2 == 0:
                        nc.vector.tensor_mul(at[0:48, j // 2, :], po[:], rb[:])
                    else:
                        ab = ab_p.tile([48, 512], F32R)
                        nc.vector.tensor_mul(ab[:], po[:], rb[:])
                        reng = (nc.gpsimd, nc.sync, nc.scalar, nc.gpsimd)[(j // 2) % 4]
                        reng.dma_start(out=at[64:112, j // 2, :], in_=ab[:])

            # normalize qp by 1/D in place (division-free attention matmuls)
            def norm_chunk(cc):
                for blk in range(NBLK):
                    bs = slice(512 * blk, 512 * (blk + 1))
                    rbig = rbig_p.tile([128, 512], F32)
                    nc.sync.dma_start(out=rbig[0:64, :],
                                      in_=_rep_row(rd_all[2 * cc:2 * cc + 1, bs], 64))
                    nc.scalar.dma_start(out=rbig[64:128, :],
                                        in_=_rep_row(rd_all[2 * cc + 1:2 * cc + 2, bs], 64))
                    nc.vector.tensor_mul(qpT[:, cc, bs], qpT[:, cc, bs], rbig[:])

            def emit_attn_head(h):
                p0 = 64 * (h % 2)
                at = at0 if h % 2 == 0 else at1
                qh = qpT[p0:p0 + 48, h // 2, :].rearrange("p (r j) -> p j r", j=8)
                for j in range(8):
                    po = po_p.tile([48, 512], F32)
                    nc.tensor.matmul(po[:], kptv_sb[p0:p0 + 48, h // 2, 0:48],
                                     qh[:, j, :], start=True, stop=True,
                                     tile_position=(p0, 0))
                    if j % 2 == 0:
                        nc.vector.tensor_copy(out=at[0:48, j // 2, :], in_=po[:])
                    else:
                        ab = ab_p.tile([48, 512], F32R)
                        nc.scalar.copy(out=ab[:], in_=po[:])
                        reng = (nc.gpsimd, nc.sync, nc.scalar, nc.gpsimd)[(j // 2) % 4]
                        reng.dma_start(out=at[64:112, j // 2, :], in_=ab[:])
                return at

            def emit_proj_head(h, at):
                for rc in range(4):
                    pz = pz_p.tile([128, C], F32)
                    for cc in range(4):
                        nc.tensor.matmul(pz[:], at[:, cc, 128 * rc:128 * (rc + 1)],
                                         wp_sb[:, cc, :],
                                         start=(cc == 0), stop=(cc == 3))
                    zo = zo_p.tile([128, C], F32)
                    if rc % 2 == 0:
                        nc.vector.tensor_copy(out=zo[:], in_=pz[:])
                    else:
                        nc.scalar.copy(out=zo[:], in_=pz[:])
                    r0 = 512 * h + 128 * rc
                    nc.sync.dma_start(out=out[r0:r0 + 128, :], in_=zo[:])

            ats = {0: at0, 1: at1}
            for cc in range(1, 4):
                norm_chunk(cc)
                for h in (2 * cc, 2 * cc + 1):
                    emit_proj_head(h - 2, ats.pop(h - 2))
                    ats[h] = emit_attn_head(h)
            emit_proj_head(6, ats.pop(6))
            emit_proj_head(7, ats.pop(7))
    nc.finalize()
    return nc


def _prep_weights(Wqkv, Wproj, bproj=None):
    """Host-side weight prep: fold dn, pad head dims, build transposed layouts."""
    Wq = Wqkv[0:C, :]
    Wk = Wqkv[C:2 * C, :]
    Wv = Wqkv[2 * C:3 * C, :]
    wq = np.zeros((C, 512), np.float32)
    for h in range(H):
        wq[:, 64 * h:64 * h + 48] = (DN * Wq[48 * h:48 * (h + 1), :]).T
    wkv = np.concatenate([(DN * Wk).T, Wv.T], axis=1).astype(np.float32)
    wp = np.zeros((512, C), np.float32)
    WprojT = Wproj.T
    for j in range(8):
        wp[64 * j:64 * j + 48, :] = WprojT[48 * j:48 * (j + 1), :]
    if bproj is not None:
        wp[63, :] = bproj
    return wq, wkv, wp


def _run(inputs, trace=False):
    from concourse.bass_utils import run_bass_kernel_spmd

    x = np.ascontiguousarray(np.asarray(inputs["x"], dtype=np.float32))
    Wqkv = np.asarray(inputs["Wqkv"], dtype=np.float32)
    Wproj = np.asarray(inputs["Wproj"], dtype=np.float32)
    bproj = np.ascontiguousarray(np.asarray(inputs["bproj"], dtype=np.float32))
    wq, wkv, wp = _prep_weights(Wqkv, Wproj, bproj)

    if "nc" not in _NC_CACHE:
        _NC_CACHE["nc"] = build_nc()
    nc = _NC_CACHE["nc"]

    ident = np.eye(128, dtype=np.float32)
    in_maps = [
        {"x": np.ascontiguousarray(x[b]), "wq": wq, "wkv": wkv, "wp": wp,
         "bias": bproj, "ident": ident}
        for b in range(B)
    ]
    res = run_bass_kernel_spmd(nc, in_maps, list(range(8)), trace=trace)
    out = np.stack([res.results[b]["out"] for b in range(B)], axis=0)
    return out, res


def kernel(**inputs) -> np.ndarray:
    out, _ = _run(inputs, trace=False)
    return out


def kernel_profiled(**inputs):
    out, res = _run(inputs, trace=True)
    return out, res
